# revision 11
# baseline (speedup 1.0000x reference)
"""Single-head causal attention (B=4, T=4096, C=1024, H=64) on 8 TRN2 cores.

Sharding: 2 cores per batch element, query rows split between the pair so
causal work is balanced. Fold 0 owns query 512-blocks starting at
{512, 1024, 2048, 3584}; fold 1 owns {0, 1536, 2560, 3072}. Grouped into 4
slots with uniform per-slot key-chunk bounds U = {8, 16, 24, 32} the SPMD
program is identical on both folds with only 8/72 wasted key-chunk
iterations per core. Causal masking is data-driven (query-index tensor vs
key indices compared on DVE), so per-core differences are input data only.

Numerics: all matmul operands bf16 with fp32 PSUM accumulation; softmax
needs no max-subtraction (|scores| <= |q||k|/8 ~ 2.6). A ones-column is
appended to v so the softmax denominator falls out of the same matmul.
Host passes x pre-transposed and pre-tiled for contiguous DMA.
"""

import numpy as np
import ml_dtypes

import concourse.bacc as bacc
import concourse.mybir as mybir
from concourse.tile import TileContext
from concourse.masks import make_identity
from concourse.bass_utils import run_bass_kernel_spmd

B, T, C, H = 4, 4096, 1024, 64
P = 128                     # SBUF partitions
NB = T // P                 # 32 key chunks of 128
CB = C // P                 # 8 contraction chunks of 128
QS = 512                    # query/projection block width
TB = T // QS                # 8 key-side projection blocks
NSLOT = 4                   # query slots per core (2048 queries)
HE = H + 1                  # v extended with a ones column (softmax denom)

FOLD_SLOT_QSTART = [
    [512, 1024, 2048, 3584],    # fold 0
    [0, 1536, 2560, 3072],      # fold 1
]
SLOT_U = [8, 16, 24, 32]        # key chunks per slot (uniform across folds)
# tks needing the data-driven causal mask (diagonal band of either fold)
SLOT_MASK_TK = [range(u - 8, u) for u in SLOT_U]

F32 = mybir.dt.float32
BF16 = mybir.dt.bfloat16
BF16NP = ml_dtypes.bfloat16


def build_bass():
    nc = bacc.Bacc("TRN2", target_bir_lowering=False, debug=False)

    x_kv_d = nc.declare_dram_parameter("x_kv", [TB, P, CB, QS], BF16, isOutput=False)
    x_q_d = nc.declare_dram_parameter("x_q", [NSLOT, P, CB, QS], BF16, isOutput=False)
    w_d = nc.declare_dram_parameter("w_all", [P, 3, CB, H], BF16, isOutput=False)
    b_d = nc.declare_dram_parameter("b_all", [H, 3], F32, isOutput=False)
    qkidx_d = nc.declare_dram_parameter(
        "qkidx", [P, NSLOT * QS + NB], F32, isOutput=False
    )
    out_d = nc.declare_dram_parameter("out", [NSLOT * QS, H], F32, isOutput=True)

    with TileContext(nc) as tc:
        with (
            tc.tile_pool(name="const", bufs=1) as const,
            tc.tile_pool(name="xio", bufs=4) as xio,
            tc.tile_pool(name="work", bufs=3) as work,
            tc.tile_pool(name="ps_s", bufs=2, space="PSUM") as ps_s,
            tc.tile_pool(name="ps_o", bufs=1, space="PSUM") as ps_o,
            tc.tile_pool(name="ps_p", bufs=2, space="PSUM") as ps_p,
            tc.tile_pool(name="ps_t", bufs=1, space="PSUM") as ps_t,
        ):
            # ---- persistent SBUF state ----
            w_sb = const.tile([P, 3, CB, H], BF16, tag="w")
            nc.sync.dma_start(w_sb[:], w_d[:])
            b_sb = const.tile([H, 3], F32, tag="b")
            nc.sync.dma_start(b_sb[:], b_d[:])
            qkidx_sb = const.tile([P, NSLOT * QS + NB], F32, tag="qkidx")
            nc.sync.dma_start(qkidx_sb[:], qkidx_d[:])
            qidx_sb = qkidx_sb[:, : NSLOT * QS]
            kidx_sb = qkidx_sb[:, NSLOT * QS :]

            id_f32 = const.tile([P, P], F32, tag="idf")
            id_bf16 = const.tile([P, P], BF16, tag="idb")
            make_identity(nc, id_f32[:])
            make_identity(nc, id_bf16[:])

            kT_sb = const.tile([H, T], BF16, tag="kT")           # [64, 4096]
            qT_sb = const.tile([H, NSLOT * QS], BF16, tag="qT")  # [64, 2048]
            vTb_sb = const.tile([H, T], BF16, tag="vTb")         # [64, 4096]
            vext_sb = const.tile([P, NB, HE], BF16, tag="vext")  # [128, 32, 65]
            nc.vector.memset(vext_sb[:, :, H:HE], 1.0)

            # ---- emission helpers (PE is in-order; emission order is the
            # static schedule). kv projections are split into small thunks so
            # they can be interleaved into the attention slots, filling the
            # PE bubbles left by the ACT exp latency. ----
            def kv_thunks(tb):
                st = {}
                cols = slice(tb * QS, (tb + 1) * QS)

                def load():
                    st["xt"] = xio.tile([P, CB, QS], BF16, tag="xt", name="xt")
                    nc.sync.dma_start(st["xt"][:], x_kv_d[tb])

                def mk_mm(which, wi, c):
                    def f():
                        if c == 0:
                            st[which] = ps_p.tile([H, QS], F32, tag="proj", name=which)
                        nc.tensor.matmul(
                            st[which][:], w_sb[:, wi, c, :], st["xt"][:, c, :],
                            start=(c == 0), stop=(c == CB - 1),
                        )
                    return f

                def k_bias():
                    nc.vector.tensor_scalar_add(
                        kT_sb[:, cols], st["kps"][:], b_sb[:, 1:2]
                    )

                def v_bias():
                    nc.vector.tensor_scalar_add(
                        vTb_sb[:, cols], st["vps"][:], b_sb[:, 2:3]
                    )

                def mk_vtr(s):
                    def f():
                        tk = tb * (QS // P) + s
                        vtp = ps_t.tile([P, H], BF16, tag="tr")
                        nc.tensor.transpose(
                            vtp[:], vTb_sb[:, tk * P : (tk + 1) * P],
                            id_bf16[:H, :H],
                        )
                        nc.vector.tensor_copy(vext_sb[:, tk, :H], vtp[:])
                    return f

                th = [load]
                th += [mk_mm("kps", 1, c) for c in range(CB)]
                th += [k_bias]
                th += [mk_mm("vps", 2, c) for c in range(CB)]
                th += [v_bias]
                th += [mk_vtr(s) for s in range(QS // P)]
                return th

            def q_proj(qb):
                xq = xio.tile([P, CB, QS], BF16, tag="xt")
                nc.sync.dma_start(xq[:], x_q_d[qb])
                qps = ps_p.tile([H, QS], F32, tag="proj")
                for c in range(CB):
                    nc.tensor.matmul(
                        qps[:], w_sb[:, 0, c, :], xq[:, c, :],
                        start=(c == 0), stop=(c == CB - 1),
                    )
                nc.vector.tensor_scalar_add(
                    qT_sb[:, qb * QS : (qb + 1) * QS], qps[:], b_sb[:, 0:1]
                )

            # keys 0..1023 must exist before slot 0 attention starts
            for th in kv_thunks(0) + kv_thunks(1):
                th()

            for slot in range(NSLOT):
                U = SLOT_U[slot]
                qcols = slice(slot * QS, (slot + 1) * QS)
                q_proj(slot)
                # kv blocks for the NEXT slot, interleaved into this one
                fill = []
                if slot < NSLOT - 1:
                    fill = kv_thunks(2 * slot + 2) + kv_thunks(2 * slot + 3)
                fi = 0

                oacc = ps_o.tile([HE, QS], F32, tag="outT")
                pipe = []  # (expT, tkp) awaiting their wv matmuls

                def emit_wv(expT, tkp):
                    for h in range(2):
                        tk = 2 * tkp + h
                        nc.tensor.matmul(
                            oacc[:], vext_sb[:, tk, :],
                            expT[:, h * QS : (h + 1) * QS],
                            start=(tk == 0), stop=(tk == U - 1),
                        )

                npairs = U // 2
                for tkp in range(npairs):
                    sps = ps_s.tile([P, 2 * QS], F32, tag="sT")
                    expT = work.tile([P, 2 * QS], BF16, tag="expT")
                    for h in range(2):
                        tk = 2 * tkp + h
                        nc.tensor.matmul(
                            sps[:, h * QS : (h + 1) * QS],
                            kT_sb[:, tk * P : (tk + 1) * P],
                            qT_sb[:, qcols], start=True, stop=True,
                        )
                    nc.scalar.activation(
                        expT[:], sps[:], mybir.ActivationFunctionType.Exp,
                        scale=float(H) ** -0.5,
                    )
                    for h in range(2):
                        tk = 2 * tkp + h
                        if tk in SLOT_MASK_TK[slot]:
                            mask = work.tile([P, QS], BF16, tag="mask")
                            nc.vector.tensor_tensor(
                                mask[:], qidx_sb[:, qcols],
                                kidx_sb[:, tk : tk + 1].to_broadcast((P, QS)),
                                mybir.AluOpType.is_ge,
                            )
                            nc.gpsimd.tensor_tensor(
                                expT[:, h * QS : (h + 1) * QS],
                                expT[:, h * QS : (h + 1) * QS],
                                mask[:], mybir.AluOpType.mult,
                            )
                    # spread the next slot's kv projections across this slot
                    want = ((tkp + 1) * len(fill) + npairs - 1) // npairs
                    while fi < min(want, len(fill)):
                        fill[fi]()
                        fi += 1
                    # wv runs one pair behind scores so PE never stalls on ACT
                    pipe.append((expT, tkp))
                    if len(pipe) > 1:
                        emit_wv(*pipe.pop(0))
                while fi < len(fill):
                    fill[fi]()
                    fi += 1
                while pipe:
                    emit_wv(*pipe.pop(0))

                oT_sb = work.tile([HE, QS], F32, tag="oT")
                nc.vector.tensor_copy(oT_sb[:], oacc[:])
                for s in range(QS // P):
                    trp = ps_t.tile([P, HE], F32, tag="tr")
                    nc.tensor.transpose(
                        trp[:], oT_sb[:, s * P : (s + 1) * P], id_f32[:HE, :HE]
                    )
                    rec = work.tile([P, 1], F32, tag="rec")
                    nc.vector.reciprocal(rec[:], trp[:, H : H + 1])
                    ofin = work.tile([P, H], F32, tag="ofin")
                    nc.vector.tensor_scalar_mul(ofin[:], trp[:, :H], rec[:])
                    row0 = slot * QS + s * P
                    nc.sync.dma_start(out_d[row0 : row0 + P, :], ofin[:])

    nc.compile()
    return nc


_NC_CACHE = None


def _get_nc():
    global _NC_CACHE
    if _NC_CACHE is None:
        _NC_CACHE = build_bass()
    return _NC_CACHE


def _tile_xT(xT_cols):
    """[C, N*512] f32 -> [N, 128, 8, 512] bf16 pre-tiled for contiguous DMA."""
    n = xT_cols.shape[1] // QS
    t = xT_cols.reshape(CB, P, n, QS)          # [co, p, tb, t]
    return np.ascontiguousarray(t.transpose(2, 1, 0, 3).astype(BF16NP))


def _core_inputs(x, Wq, bq, Wk, bk, Wv, bv, b, fold):
    xT = np.asarray(x[b], dtype=np.float32).T  # [C, T] (view)
    qstarts = FOLD_SLOT_QSTART[fold]
    qcols = np.concatenate([np.arange(q0, q0 + QS) for q0 in qstarts])
    w_all = np.stack(
        [np.asarray(w, np.float32).reshape(CB, P, H) for w in (Wq, Wk, Wv)], axis=1
    )  # [co, 3, p, h]
    w_all = np.ascontiguousarray(w_all.transpose(2, 1, 0, 3).astype(BF16NP))
    b_all = np.ascontiguousarray(
        np.stack([np.asarray(v, np.float32) for v in (bq, bk, bv)], axis=1)
    )
    qidx = np.broadcast_to(qcols.astype(np.float32)[None, :], (P, NSLOT * QS))
    kidx = (
        np.arange(NB, dtype=np.float32)[None, :] * P
        + np.arange(P, dtype=np.float32)[:, None]
    )
    qkidx = np.ascontiguousarray(
        np.concatenate([qidx, kidx], axis=1, dtype=np.float32)
    )
    return {
        "x_kv": _tile_xT(xT),
        "x_q": _tile_xT(xT[:, qcols]),
        "w_all": w_all,
        "b_all": b_all,
        "qkidx": qkidx,
    }


def kernel(x, Wq, bq, Wk, bk, Wv, bv):
    x = np.asarray(x, dtype=np.float32)
    nc = _get_nc()
    core_ids = list(range(8))
    in_maps = [
        _core_inputs(x, Wq, bq, Wk, bk, Wv, bv, core // 2, core % 2)
        for core in core_ids
    ]
    res = run_bass_kernel_spmd(nc, in_maps, core_ids)
    out = np.empty((B, T, H), dtype=np.float32)
    for core in core_ids:
        b, fold = core // 2, core % 2
        co = res.results[core]["out"]  # [2048, 64]
        for slot, q0 in enumerate(FOLD_SLOT_QSTART[fold]):
            out[b, q0 : q0 + QS, :] = co[slot * QS : (slot + 1) * QS, :]
    return out


# revision 12
# speedup vs baseline: 1.2217x; 1.2217x over previous
"""Single-head causal attention (B=4, T=4096, C=1024, H=64) on 8 TRN2 cores.

Sharding: 2 cores per batch element, query rows split between the pair so
causal work is balanced. Fold 0 owns query 512-blocks starting at
{512, 1024, 2048, 3584}; fold 1 owns {0, 1536, 2560, 3072}. Grouped into 4
slots with uniform per-slot key-chunk bounds U = {8, 16, 24, 32} the SPMD
program is identical on both folds with only 8/72 wasted key-chunk
iterations per core. Causal masking is data-driven (query-index tensor vs
key indices compared on DVE), so per-core differences are input data only.

Numerics: all matmul operands bf16 with fp32 PSUM accumulation; softmax
needs no max-subtraction (|scores| <= |q||k|/8 ~ 2.6). A ones-column is
appended to v so the softmax denominator falls out of the same matmul.
Host passes x pre-transposed and pre-tiled for contiguous DMA.
"""

import numpy as np
import ml_dtypes

import concourse.bacc as bacc
import concourse.mybir as mybir
from concourse.tile import TileContext
from concourse.masks import make_identity
from concourse.bass_utils import run_bass_kernel_spmd

B, T, C, H = 4, 4096, 1024, 64
P = 128                     # SBUF partitions
NB = T // P                 # 32 key chunks of 128
CB = C // P                 # 8 contraction chunks of 128
QS = 512                    # query/projection block width
TB = T // QS                # 8 key-side projection blocks
NSLOT = 4                   # query slots per core (2048 queries)
HE = H + 1                  # v extended with a ones column (softmax denom)

FOLD_SLOT_QSTART = [
    [512, 1024, 2048, 3584],    # fold 0
    [0, 1536, 2560, 3072],      # fold 1
]
SLOT_U = [8, 16, 24, 32]        # key chunks per slot (uniform across folds)
# tks needing the data-driven causal mask (diagonal band of either fold)
SLOT_MASK_TK = [range(u - 8, u) for u in SLOT_U]

F32 = mybir.dt.float32
BF16 = mybir.dt.bfloat16
BF16NP = ml_dtypes.bfloat16


def build_bass():
    nc = bacc.Bacc("TRN2", target_bir_lowering=False, debug=False)

    x_kv_d = nc.declare_dram_parameter("x_kv", [TB, P, CB, QS], BF16, isOutput=False)
    x_q_d = nc.declare_dram_parameter("x_q", [NSLOT, P, CB, QS], BF16, isOutput=False)
    w_d = nc.declare_dram_parameter("w_all", [P, 3, CB, H], BF16, isOutput=False)
    b_d = nc.declare_dram_parameter("b_all", [H, 3], F32, isOutput=False)
    qkidx_d = nc.declare_dram_parameter(
        "qkidx", [P, NSLOT * QS + NB], F32, isOutput=False
    )
    out_d = nc.declare_dram_parameter("out", [NSLOT * QS, H], F32, isOutput=True)

    with TileContext(nc) as tc:
        with (
            tc.tile_pool(name="const", bufs=1) as const,
            tc.tile_pool(name="xio", bufs=4) as xio,
            tc.tile_pool(name="work", bufs=3) as work,
            tc.tile_pool(name="ps_s", bufs=2, space="PSUM") as ps_s,
            tc.tile_pool(name="ps_o", bufs=1, space="PSUM") as ps_o,
            tc.tile_pool(name="ps_p", bufs=2, space="PSUM") as ps_p,
            tc.tile_pool(name="ps_t", bufs=1, space="PSUM") as ps_t,
        ):
            # ---- persistent SBUF state ----
            w_sb = const.tile([P, 3, CB, H], BF16, tag="w")
            nc.sync.dma_start(w_sb[:], w_d[:])
            b_sb = const.tile([H, 3], F32, tag="b")
            nc.sync.dma_start(b_sb[:], b_d[:])
            qkidx_sb = const.tile([P, NSLOT * QS + NB], F32, tag="qkidx")
            nc.sync.dma_start(qkidx_sb[:], qkidx_d[:])
            qidx_sb = qkidx_sb[:, : NSLOT * QS]
            kidx_sb = qkidx_sb[:, NSLOT * QS :]

            id_f32 = const.tile([P, P], F32, tag="idf")
            id_bf16 = const.tile([P, P], BF16, tag="idb")
            make_identity(nc, id_f32[:])
            make_identity(nc, id_bf16[:])

            kT_sb = const.tile([H, T], BF16, tag="kT")           # [64, 4096]
            qT_sb = const.tile([H, NSLOT * QS], BF16, tag="qT")  # [64, 2048]
            vTb_sb = const.tile([H, T], BF16, tag="vTb")         # [64, 4096]
            vext_sb = const.tile([P, NB, HE], BF16, tag="vext")  # [128, 32, 65]
            nc.vector.memset(vext_sb[:, :, H:HE], 1.0)

            # ---- emission helpers (PE is in-order; emission order is the
            # static schedule). kv projections are split into small thunks so
            # they can be interleaved into the attention slots, filling the
            # PE bubbles left by the ACT exp latency. ----
            def kv_thunks(tb):
                st = {}
                cols = slice(tb * QS, (tb + 1) * QS)

                def load():
                    st["xt"] = xio.tile([P, CB, QS], BF16, tag="xt", name="xt")
                    nc.sync.dma_start(st["xt"][:], x_kv_d[tb])

                def mk_mm(which, wi, c):
                    def f():
                        if c == 0:
                            st[which] = ps_p.tile([H, QS], F32, tag="proj", name=which)
                        nc.tensor.matmul(
                            st[which][:], w_sb[:, wi, c, :], st["xt"][:, c, :],
                            start=(c == 0), stop=(c == CB - 1),
                        )
                    return f

                def k_bias():
                    nc.vector.tensor_scalar_add(
                        kT_sb[:, cols], st["kps"][:], b_sb[:, 1:2]
                    )

                def v_bias():
                    nc.vector.tensor_scalar_add(
                        vTb_sb[:, cols], st["vps"][:], b_sb[:, 2:3]
                    )

                def mk_vtr(s):
                    def f():
                        tk = tb * (QS // P) + s
                        vtp = ps_t.tile([P, H], BF16, tag="tr")
                        nc.tensor.transpose(
                            vtp[:], vTb_sb[:, tk * P : (tk + 1) * P],
                            id_bf16[:H, :H],
                        )
                        nc.vector.tensor_copy(vext_sb[:, tk, :H], vtp[:])
                    return f

                th = [load]
                th += [mk_mm("kps", 1, c) for c in range(CB)]
                th += [k_bias]
                th += [mk_mm("vps", 2, c) for c in range(CB)]
                th += [v_bias]
                th += [mk_vtr(s) for s in range(QS // P)]
                return th

            def q_proj(qb):
                xq = xio.tile([P, CB, QS], BF16, tag="xt")
                nc.sync.dma_start(xq[:], x_q_d[qb])
                qps = ps_p.tile([H, QS], F32, tag="proj")
                for c in range(CB):
                    nc.tensor.matmul(
                        qps[:], w_sb[:, 0, c, :], xq[:, c, :],
                        start=(c == 0), stop=(c == CB - 1),
                    )
                nc.vector.tensor_scalar_add(
                    qT_sb[:, qb * QS : (qb + 1) * QS], qps[:], b_sb[:, 0:1]
                )

            # keys 0..1023 must exist before slot 0 attention starts
            for th in kv_thunks(0) + kv_thunks(1):
                th()

            for slot in range(NSLOT):
                U = SLOT_U[slot]
                qcols = slice(slot * QS, (slot + 1) * QS)
                q_proj(slot)
                # kv blocks for the NEXT slot, interleaved into this one
                fill = []
                if slot < NSLOT - 1:
                    fill = kv_thunks(2 * slot + 2) + kv_thunks(2 * slot + 3)
                fi = 0

                oacc = ps_o.tile([HE, QS], F32, tag="outT")
                pipe = []  # (expT, tkp) awaiting their wv matmuls

                def emit_wv(expT, tkp):
                    for h in range(2):
                        tk = 2 * tkp + h
                        nc.tensor.matmul(
                            oacc[:], vext_sb[:, tk, :],
                            expT[:, h * QS : (h + 1) * QS],
                            start=(tk == 0), stop=(tk == U - 1),
                        )

                npairs = U // 2
                for tkp in range(npairs):
                    sps = ps_s.tile([P, 2 * QS], F32, tag="sT")
                    expT = work.tile([P, 2 * QS], BF16, tag="expT")
                    for h in range(2):
                        tk = 2 * tkp + h
                        nc.tensor.matmul(
                            sps[:, h * QS : (h + 1) * QS],
                            kT_sb[:, tk * P : (tk + 1) * P],
                            qT_sb[:, qcols], start=True, stop=True,
                        )
                    nc.scalar.activation(
                        expT[:], sps[:], mybir.ActivationFunctionType.Exp,
                        scale=float(H) ** -0.5,
                    )
                    for h in range(2):
                        tk = 2 * tkp + h
                        if tk in SLOT_MASK_TK[slot]:
                            mask = work.tile([P, QS], BF16, tag="mask")
                            nc.vector.tensor_tensor(
                                mask[:], qidx_sb[:, qcols],
                                kidx_sb[:, tk : tk + 1].to_broadcast((P, QS)),
                                mybir.AluOpType.is_ge,
                            )
                            nc.vector.tensor_tensor(
                                expT[:, h * QS : (h + 1) * QS],
                                expT[:, h * QS : (h + 1) * QS],
                                mask[:], mybir.AluOpType.mult,
                            )
                    # spread the next slot's kv projections across this slot
                    want = ((tkp + 1) * len(fill) + npairs - 1) // npairs
                    while fi < min(want, len(fill)):
                        fill[fi]()
                        fi += 1
                    # wv runs one pair behind scores so PE never stalls on ACT
                    pipe.append((expT, tkp))
                    if len(pipe) > 1:
                        emit_wv(*pipe.pop(0))
                while fi < len(fill):
                    fill[fi]()
                    fi += 1
                while pipe:
                    emit_wv(*pipe.pop(0))

                oT_sb = work.tile([HE, QS], F32, tag="oT")
                nc.vector.tensor_copy(oT_sb[:], oacc[:])
                for s in range(QS // P):
                    trp = ps_t.tile([P, HE], F32, tag="tr")
                    nc.tensor.transpose(
                        trp[:], oT_sb[:, s * P : (s + 1) * P], id_f32[:HE, :HE]
                    )
                    rec = work.tile([P, 1], F32, tag="rec")
                    nc.vector.reciprocal(rec[:], trp[:, H : H + 1])
                    ofin = work.tile([P, H], F32, tag="ofin")
                    nc.vector.tensor_scalar_mul(ofin[:], trp[:, :H], rec[:])
                    row0 = slot * QS + s * P
                    nc.sync.dma_start(out_d[row0 : row0 + P, :], ofin[:])

    nc.compile()
    return nc


_NC_CACHE = None


def _get_nc():
    global _NC_CACHE
    if _NC_CACHE is None:
        _NC_CACHE = build_bass()
    return _NC_CACHE


def _tile_xT(xT_cols):
    """[C, N*512] f32 -> [N, 128, 8, 512] bf16 pre-tiled for contiguous DMA."""
    n = xT_cols.shape[1] // QS
    t = xT_cols.reshape(CB, P, n, QS)          # [co, p, tb, t]
    return np.ascontiguousarray(t.transpose(2, 1, 0, 3).astype(BF16NP))


def _core_inputs(x, Wq, bq, Wk, bk, Wv, bv, b, fold):
    xT = np.asarray(x[b], dtype=np.float32).T  # [C, T] (view)
    qstarts = FOLD_SLOT_QSTART[fold]
    qcols = np.concatenate([np.arange(q0, q0 + QS) for q0 in qstarts])
    w_all = np.stack(
        [np.asarray(w, np.float32).reshape(CB, P, H) for w in (Wq, Wk, Wv)], axis=1
    )  # [co, 3, p, h]
    w_all = np.ascontiguousarray(w_all.transpose(2, 1, 0, 3).astype(BF16NP))
    b_all = np.ascontiguousarray(
        np.stack([np.asarray(v, np.float32) for v in (bq, bk, bv)], axis=1)
    )
    qidx = np.broadcast_to(qcols.astype(np.float32)[None, :], (P, NSLOT * QS))
    kidx = (
        np.arange(NB, dtype=np.float32)[None, :] * P
        + np.arange(P, dtype=np.float32)[:, None]
    )
    qkidx = np.ascontiguousarray(
        np.concatenate([qidx, kidx], axis=1, dtype=np.float32)
    )
    return {
        "x_kv": _tile_xT(xT),
        "x_q": _tile_xT(xT[:, qcols]),
        "w_all": w_all,
        "b_all": b_all,
        "qkidx": qkidx,
    }


def kernel(x, Wq, bq, Wk, bk, Wv, bv):
    x = np.asarray(x, dtype=np.float32)
    nc = _get_nc()
    core_ids = list(range(8))
    in_maps = [
        _core_inputs(x, Wq, bq, Wk, bk, Wv, bv, core // 2, core % 2)
        for core in core_ids
    ]
    res = run_bass_kernel_spmd(nc, in_maps, core_ids)
    out = np.empty((B, T, H), dtype=np.float32)
    for core in core_ids:
        b, fold = core // 2, core % 2
        co = res.results[core]["out"]  # [2048, 64]
        for slot, q0 in enumerate(FOLD_SLOT_QSTART[fold]):
            out[b, q0 : q0 + QS, :] = co[slot * QS : (slot + 1) * QS, :]
    return out
